# revision 37
# baseline (speedup 1.0000x reference)
"""Trainium2 Bass kernel for nn_MHA_2516850835986.

MHA: B=1, T=2048, C=2048, H=32 heads, d=64, causal, RoPE (head-indexed
angle quirk: within head h all feature pairs rotate by t * 10000^(-h/32)).

Sharding: head-parallel across 8 cores (4 heads each). x is replicated
(pre-transposed + pre-tiled on host), qkv columns / proj rows sharded by
head. Each core produces a partial [T, C] output (proj contraction over
its own heads' features) in bf16; partials are summed on host in f32.

v2 (from 409us baseline):
- bf16 operands everywhere (same 1 cycle/row PE rate as f32r at wide
  moving dims, but half the HBM traffic, half the SBUF footprint, and no
  f32r 4x penalty on narrow moving dims).
- software pipelining: tile i's proj + softmax-normalize tails are
  emitted AFTER tile i+1's qkv/rope matmuls, so the PE never waits on
  the reciprocal chain.
- reciprocal_approx_fast (~5x faster than reciprocal, 18 good bits).
- causal triangle: diagonal 128-blocks restrict S/exp/PV to t >= 128*b.
- RoPE cos-multiply moved to GpSimd; per-core layout pre-tiled on host
  so every input DMA is contiguous per partition.
"""

import sys

sys.path.insert(0, "/opt/trn_rl_repo")
import numpy as np

T = 2048
C = 2048
NH = 32          # total heads
HL = 4           # heads per core
D = 64           # head dim
NC_ = 8          # cores
TT = 512         # t-tile width
NTT = T // TT    # 4 t-tiles
KC = C // 128    # 16 contraction chunks (also 16 s-blocks of 128)
ROPE_THETA = 10000.0

_CACHE = {}


def _build_program():
    import concourse.bass as bass
    import concourse.tile as tile
    from concourse import bacc, mybir
    from contextlib import ExitStack

    F32 = mybir.dt.float32
    F32R = mybir.dt.float32r
    BF16 = mybir.dt.bfloat16
    EXP = mybir.ActivationFunctionType.Exp
    MUL = mybir.AluOpType.mult
    ADD = mybir.AluOpType.add

    nc = bacc.Bacc(None, target_bir_lowering=False)

    # host-pretiled x^T: [p, tile, kc, t'] so tile loads are contiguous
    xt = nc.declare_dram_parameter("xt", [128, NTT, KC, TT], BF16, False)
    wqk = nc.declare_dram_parameter("wqk", [128, KC, 512], BF16, False)
    wv = nc.declare_dram_parameter("wv", [128, KC, 256], BF16, False)
    wproj = nc.declare_dram_parameter("wproj", [128, 2, T], BF16, False)
    costab = nc.declare_dram_parameter("costab", [128, 2, T], BF16, False)
    sintab = nc.declare_dram_parameter("sintab", [128, 2, T], BF16, False)
    tri = nc.declare_dram_parameter("tri", [128, 4, TT], BF16, False)
    perm = nc.declare_dram_parameter("perm", [128, 128], F32R, False)
    eye = nc.declare_dram_parameter("eye", [64, 64], BF16, False)
    out = nc.declare_dram_parameter("out", [T, T], BF16, True)

    with tile.TileContext(nc) as tc, ExitStack() as ctx:
        consts = ctx.enter_context(tc.tile_pool(name="consts", bufs=1))
        xtp = ctx.enter_context(tc.tile_pool(name="xtp", bufs=8))
        csp = ctx.enter_context(tc.tile_pool(name="csp", bufs=4))
        qrawp = ctx.enter_context(tc.tile_pool(name="qrawp", bufs=1))
        csfp = ctx.enter_context(tc.tile_pool(name="csfp", bufs=1))
        rscp = ctx.enter_context(tc.tile_pool(name="rscp", bufs=2))
        qrotp = ctx.enter_context(tc.tile_pool(name="qrotp", bufs=2))
        persist = ctx.enter_context(tc.tile_pool(name="persist", bufs=1))
        p4p = ctx.enter_context(tc.tile_pool(name="p4p", bufs=2))
        ytp = ctx.enter_context(tc.tile_pool(name="ytp", bufs=2))
        ytmpp = ctx.enter_context(tc.tile_pool(name="ytmpp", bufs=2))
        ymp = ctx.enter_context(tc.tile_pool(name="ymp", bufs=4))
        rp = ctx.enter_context(tc.tile_pool(name="rp", bufs=1))
        outp = ctx.enter_context(tc.tile_pool(name="outp", bufs=8))

        # PSUM: S2 pairs / qk accum (2 banks x2) + y (1 bank x2) + misc (1 bank x2)
        sps = ctx.enter_context(tc.tile_pool(name="sps", bufs=2, space="PSUM"))
        yps = ctx.enter_context(tc.tile_pool(name="yps", bufs=2, space="PSUM"))
        unips = ctx.enter_context(tc.tile_pool(name="unips", bufs=2, space="PSUM"))

        wqk_sb = consts.tile([128, KC, 512], BF16)
        wv_sb = consts.tile([128, KC, 256], BF16)
        wproj_sb = consts.tile([128, 2, T], BF16)
        tri_sb = consts.tile([128, 4, TT], BF16)
        perm_sb = consts.tile([128, 128], F32R)
        eye_sb = consts.tile([64, 64], BF16)
        ones_sb = consts.tile([1, 64], F32R)
        nc.vector.memset(ones_sb[:].bitcast(F32), 1.0)

        # v in normal layout [s, dd]: per s-block slot of 4 heads x (64 v + 1 one + 1 pad)
        v_sb = persist.tile([128, KC, HL, 66], BF16)
        # fill everything with 1.0 once; v-copies overwrite cols 0:64 of each
        # slot, leaving col 64 as the ones-column for the denominator trick
        nc.vector.memset(v_sb[:].rearrange("p a b c -> p (a b c)"), 1.0)
        # k^T (rope'd), persistent across tiles: [dd(2 heads), block, t]
        krot = persist.tile([128, 2, T], BF16)

        def load_tile(j):
            """Issue input DMAs for t-tile j (sync HWDGE queue only).

            cos/sin arrive bf16 and are cast to f32 working tiles at use
            time (in qkvrope) to keep RoPE math in f32."""
            tslj = slice(TT * j, TT * (j + 1))
            xth = []
            for half in range(2):
                xh = xtp.tile([128, KC // 2, TT], BF16, tag="xt")
                nc.sync.dma_start(xh[:], xt[:, j, (KC // 2) * half:(KC // 2) * (half + 1), :])
                xth.append(xh)
            cos_t = csp.tile([128, 2, TT], BF16, tag="cos")
            nc.sync.dma_start(cos_t[:], costab[:, :, tslj])
            sin_t = csp.tile([128, 2, TT], BF16, tag="sin")
            nc.sync.dma_start(sin_t[:], sintab[:, :, tslj])
            return xth, cos_t, sin_t

        # tile-0 inputs interleaved with the wqk chunks so the first qk
        # chain can start after ~0.25MB instead of the whole preamble
        xh0 = xtp.tile([128, KC // 2, TT], BF16, tag="xt")
        xh1 = xtp.tile([128, KC // 2, TT], BF16, tag="xt")
        xhv = [xh0, xh1]
        for lo, hi in ((0, 2), (2, 4), (4, 8), (8, 12), (12, 16)):
            nc.sync.dma_start(wqk_sb[:, lo:hi, :], wqk[:, lo:hi, :])
            nc.sync.dma_start(xhv[lo // 8][:, lo % 8:lo % 8 + (hi - lo), :],
                              xt[:, 0, lo:hi, :])
        cos0 = csp.tile([128, 2, TT], BF16, tag="cos")
        nc.sync.dma_start(cos0[:], costab[:, :, 0:TT])
        sin0 = csp.tile([128, 2, TT], BF16, tag="sin")
        nc.sync.dma_start(sin0[:], sintab[:, :, 0:TT])
        nc.sync.dma_start(wv_sb[:], wv[:])
        nc.sync.dma_start(perm_sb[:], perm[:])
        nc.sync.dma_start(eye_sb[:], eye[:])
        nc.sync.dma_start(tri_sb[:], tri[:])
        # preload ALL remaining tiles' inputs up front (fits in SBUF at
        # bf16); ordered by first-use time, wproj between xt1 and xt2.
        # Steady state then has NO input DMA bursts competing with the
        # latency-critical yt-swap SBUF DMAs at tile boundaries.
        loads = [([xh0, xh1], cos0, sin0)]
        loads.append(load_tile(1))
        nc.sync.dma_start(wproj_sb[:], wproj[:])
        loads.append(load_tile(2))
        loads.append(load_tile(3))

        def qkvrope(i, xth, cos_t, sin_t):
            """qk/v matmuls + RoPE for tile i. Returns qrot; writes krot, v_sb.

            qk accumulation is quarter-interleaved (qq outer) so tile-0's
            first matmuls only wait on the first wqk/xt quarter DMA pair.
            The 4 m-psums live in the two 2-bank sps tiles as halves."""
            tsl = slice(TT * i, TT * (i + 1))
            psA = sps.tile([128, 2 * TT], F32, tag="S")
            psB = sps.tile([128, 2 * TT], F32, tag="S")
            mslot = [psA[:, 0:TT], psA[:, TT:2 * TT],
                     psB[:, 0:TT], psB[:, TT:2 * TT]]
            for qq in range(4):
                for m in range(4):
                    for kc in range(4 * qq, 4 * (qq + 1)):
                        nc.tensor.matmul(mslot[m],
                                         wqk_sb[:, kc, 128 * m:128 * (m + 1)],
                                         xth[kc // 8][:, kc % 8, :],
                                         start=(kc == 0), stop=(kc == KC - 1))
            qraw = qrawp.tile([128, 4, TT], F32R, tag="qraw")
            for m in range(4):
                nc.vector.tensor_copy(qraw[:, m, :], mslot[m])

            # cast bf16 cos/sin to f32 working tiles (keeps RoPE math f32);
            # on DVE — gpsimd CAST is ~5x slower and thrashes its library
            cs32 = csfp.tile([128, 4, TT], F32, tag="cs32")
            nc.vector.tensor_copy(cs32[:, 0:2, :], cos_t[:])
            nc.vector.tensor_copy(cs32[:, 2:4, :], sin_t[:])

            # ---- v matmuls (normal layout) ----
            # before RoPE so the PE's v block hides the qraw-evacuation and
            # cos/sin-cast DVE latency feeding the perm matmuls
            for tc4 in range(4):
                psv = unips.tile([128, TT], F32, tag="uni")
                for kc in range(KC):
                    nc.tensor.matmul(psv[:, 0:256],
                                     xth[kc // 8][:, kc % 8, 128 * tc4:128 * (tc4 + 1)],
                                     wv_sb[:, kc, :],
                                     start=(kc == 0), stop=(kc == KC - 1))
                nc.vector.tensor_copy(
                    v_sb[:, 4 * i + tc4, :, 0:64],
                    psv[:, 0:256].rearrange("p (h d) -> p h d", h=HL))

            # ---- RoPE on q (blocks 0,1) and k (blocks 2,3) ----
            # dst = cos*src + sin*(perm@src); cos-mul on GpSimd, rest DVE
            qrot = qrotp.tile([128, 2, TT], BF16, tag="qrot")
            for bb in range(4):
                blk = bb % 2
                src = qraw[:, bb, :]
                dst = qrot[:, blk, :] if bb < 2 else krot[:, blk, tsl]
                psw = unips.tile([128, TT], F32, tag="uni")
                nc.tensor.matmul(psw[:], perm_sb[:], src, start=True, stop=True)
                nc.vector.tensor_tensor(psw[:], psw[:], cs32[:, 2 + blk, :], MUL)
                t1 = rscp.tile([128, TT], F32, tag="rsc")
                nc.gpsimd.tensor_tensor(t1[:], src.bitcast(F32), cs32[:, blk, :], MUL)
                nc.vector.tensor_tensor(dst, t1[:], psw[:], ADD)
            return qrot

        def finish_tails(yt, tails):
            """Broadcast each head's 1/denominator across 64 partitions via
            PE, normalize into yt (bf16); hh=1 is lifted onto partitions
            64:128 by an identity matmul into PSUM (no DMA — stays on the
            engines, so proj's dependency chain is short and local)."""
            for bp, hh, ym64, rsb in tails:
                psb = yps.tile([128, TT], F32, tag="y")
                nc.tensor.matmul(psb[0:64, :], ones_sb[:], rsb[:],
                                 start=True, stop=True)
                if hh == 0:
                    nc.vector.tensor_tensor(yt[0:64, bp, :], ym64[:],
                                            psb[0:64, :], MUL)
                else:
                    ytm = ytmpp.tile([64, TT], BF16, tag="ytmp2")
                    nc.vector.tensor_tensor(ytm[:], ym64[:], psb[0:64, :], MUL)
                    pmv = yps.tile([128, TT], F32, tag="y")
                    nc.tensor.matmul(pmv[64:128, :], eye_sb[:], ytm[:],
                                     start=True, stop=True)
                    nc.vector.tensor_copy(yt[64:128, bp, :], pmv[64:128, :])

        def attention(i, qrot):
            """S -> exp -> (mask) -> PV per head pair; evacuate y + start
            reciprocals. bp0's broadcast+normalize tails are emitted at the
            end (hidden behind bp1's matmuls); bp1's are returned for
            deferred emission (after the next tile's qkvrope)."""
            yt = ytp.tile([128, 2, TT], BF16, tag="yt")
            nsb = 4 * (i + 1)
            tails = []
            for bp in range(2):
                psyA = yps.tile([65, TT], F32, tag="y")
                psyB = yps.tile([65, TT], F32, tag="y")

                def emit_pv(sb, p4):
                    diag = sb - 4 * i
                    c0 = 128 * diag if diag > 0 else 0
                    nc.tensor.matmul(psyA[:, c0:TT], v_sb[:, sb, 2 * bp, 0:65],
                                     p4[:, c0:TT],
                                     start=(sb == 0), stop=(sb == nsb - 1))
                    nc.tensor.matmul(psyB[:, c0:TT], v_sb[:, sb, 2 * bp + 1, 0:65],
                                     p4[:, TT + c0:2 * TT],
                                     start=(sb == 0), stop=(sb == nsb - 1))

                prev = None   # (sb, p4) — PV trails S by one block to hide exp
                for sb in range(nsb):
                    diag = sb - 4 * i        # >= 0 on the diagonal t-tile
                    c0 = 128 * diag if diag > 0 else 0
                    s2 = sps.tile([128, 2 * TT], F32, tag="S")
                    nc.tensor.matmul(s2[:, c0:TT],
                                     krot[0:64, bp, 128 * sb:128 * (sb + 1)],
                                     qrot[0:64, bp, c0:TT],
                                     start=True, stop=True, tile_position=(0, 0))
                    nc.tensor.matmul(s2[:, TT + c0:2 * TT],
                                     krot[64:128, bp, 128 * sb:128 * (sb + 1)],
                                     qrot[64:128, bp, c0:TT],
                                     start=True, stop=True, tile_position=(64, 0))
                    p4 = p4p.tile([128, 2 * TT], BF16, tag="P4")
                    if c0 == 0:
                        nc.scalar.activation(p4[:], s2[:], EXP, scale=0.125)
                    else:
                        nc.scalar.activation(p4[:, c0:TT], s2[:, c0:TT],
                                             EXP, scale=0.125)
                        nc.scalar.activation(p4[:, TT + c0:2 * TT],
                                             s2[:, TT + c0:2 * TT],
                                             EXP, scale=0.125)
                    if diag >= 0:
                        # mask only the partial-triangle 128-col slab
                        for hh in range(2):
                            off = TT * hh + c0
                            nc.gpsimd.tensor_tensor(
                                p4[:, off:off + 128],
                                p4[:, off:off + 128],
                                tri_sb[:, diag, c0:c0 + 128], MUL)
                    if prev is not None:
                        emit_pv(*prev)
                    prev = (sb, p4)
                emit_pv(*prev)
                for hh, psy in ((0, psyA), (1, psyB)):
                    ym64 = ymp.tile([64, TT], F32, tag="ym")
                    nc.scalar.copy(ym64[:], psy[0:64, :])
                    # denominator row evacuated to partition 0 so the fast
                    # approx reciprocal works (it mishandles offset inputs)
                    dn0 = rp.tile([1, TT], F32, tag="dn", bufs=2)
                    nc.scalar.copy(dn0[:], psy[64:65, :])
                    rsf = rp.tile([1, TT], F32, tag="rsf", bufs=2)
                    rsb = rp.tile([1, TT], F32R, tag=f"r{bp}{hh}")
                    with nc.allow_low_precision("softmax recip approx+f32r"):
                        nc.vector.reciprocal_approx_fast(out=rsf[:], in_=dn0[:])
                        nc.vector.tensor_copy(rsb[:], rsf[:])
                    tails.append((bp, hh, ym64, rsb))
            # bp0's tails now: its reciprocals completed during bp1's matmuls
            finish_tails(yt, tails[0:2])
            return yt, tails[2:4]

        def emit_proj(j, ytj):
            """Partial out rows for t-tile j from its normalized y^T.
            Out DMAs round-robin over 3 queues (sync joins once the input
            preloads are done after tile 0)."""
            qs = ([nc.scalar.dma_start, nc.gpsimd.dma_start] if j == 0 else
                  [nc.scalar.dma_start, nc.gpsimd.dma_start,
                   nc.sync.dma_start])
            n = 0
            for tc4 in range(4):
                for ct in range(4):
                    pso = yps.tile([128, TT], F32, tag="y")
                    for b in range(2):
                        nc.tensor.matmul(pso[:],
                                         ytj[:, b, 128 * tc4:128 * (tc4 + 1)],
                                         wproj_sb[:, b, TT * ct:TT * (ct + 1)],
                                         start=(b == 0), stop=(b == 1))
                    osb = outp.tile([128, TT], BF16, tag="osb")
                    if ct % 2 == 0:
                        nc.scalar.copy(osb[:], pso[:])
                    else:
                        nc.vector.tensor_copy(osb[:], pso[:])
                    dst = out[TT * j + 128 * tc4: TT * j + 128 * (tc4 + 1),
                              TT * ct:TT * (ct + 1)]
                    qs[n % len(qs)](dst, osb[:])
                    n += 1

        qrots = {0: qkvrope(0, *loads[0])}
        for i in range(NTT):
            yt, tails = attention(i, qrots.pop(i))
            if i + 1 < NTT:
                qrots[i + 1] = qkvrope(i + 1, *loads[i + 1])
            finish_tails(yt, tails)
            emit_proj(i, yt)

    nc.finalize()
    return nc


def _host_inputs(x, w_qkv, w_proj, attn_mask):
    """Build the 8 per-core input maps (host-side sharding/layout prep)."""
    import ml_dtypes
    BF = ml_dtypes.bfloat16

    x = np.asarray(x)
    w_qkv = np.asarray(w_qkv)
    w_proj = np.asarray(w_proj)
    attn_mask = np.asarray(attn_mask)

    xT = x.reshape(T, C).T                               # [C, T] f32
    # pre-tiled [p, tile, kc, t'] contiguous bf16
    xtt = xT.reshape(KC, 128, NTT, TT).transpose(1, 2, 0, 3).astype(BF)

    # RoPE tables, faithful to the reference broadcasting quirk:
    # head g rotates all pairs by angle t * theta^(-g/32) (f32 math).
    inv_freq = (1.0 / (ROPE_THETA ** (np.arange(0, D, 2, dtype=np.float32) / D))
                ).astype(np.float32)                     # [32] indexed by head
    t_ar = np.arange(T, dtype=np.float32)
    freqs = (t_ar[:, None] * inv_freq[None, :]).astype(np.float32)  # [T, 32]
    cosf = np.cos(freqs).astype(np.float32)              # [T, 32]
    sinf = np.sin(freqs).astype(np.float32)
    sgn = np.where(np.arange(64) % 2 == 0, np.float32(-1.0), np.float32(1.0))  # [64]

    # 0/1 causal keep-masks from the actual mask, one per 128-row s-block of a
    # 512-wide diagonal t-tile: tri[s, b, t] = exp(mask[t, 128b + s])
    tri = np.empty((128, 4, TT), dtype=np.float32)
    for bq in range(4):
        tri[:, bq, :] = np.exp(
            attn_mask[0:TT, 128 * bq:128 * (bq + 1)].astype(np.float64)).T
    tri = tri.astype(BF)

    permM = np.zeros((128, 128), dtype=np.float32)
    permM[np.arange(128), np.arange(128) ^ 1] = 1.0

    in_maps = []
    for c in range(NC_):
        wqk_c = np.concatenate(
            [w_qkv[:, 256 * c:256 * (c + 1)],
             w_qkv[:, 2048 + 256 * c:2048 + 256 * (c + 1)]], axis=1)  # [C, 512]
        wqk_c = wqk_c.reshape(KC, 128, 512).transpose(1, 0, 2).astype(BF)
        wv_c = w_qkv[:, 4096 + 256 * c:4096 + 256 * (c + 1)]          # [C, 256]
        wv_c = wv_c.reshape(KC, 128, 256).transpose(1, 0, 2).astype(BF)
        wproj_c = w_proj[256 * c:256 * (c + 1), :]                    # [256, T]
        wproj_c = wproj_c.reshape(2, 128, T).transpose(1, 0, 2).astype(BF)

        costab = np.empty((128, 2, T), dtype=np.float32)
        sintab = np.empty((128, 2, T), dtype=np.float32)
        for bb in range(2):
            for p in range(128):
                g = 4 * c + 2 * bb + (p // 64)           # global head
                costab[p, bb, :] = cosf[:, g]
                sintab[p, bb, :] = sgn[p % 64] * sinf[:, g]
        costab = costab.astype(BF)
        sintab = sintab.astype(BF)

        in_maps.append({
            "xt": xtt, "wqk": wqk_c, "wv": wv_c, "wproj": wproj_c,
            "costab": costab, "sintab": sintab, "tri": tri, "perm": permM,
            "eye": np.eye(64, dtype=np.float32).astype(BF),
        })
    return in_maps


def _get_program():
    if "nc" not in _CACHE:
        _CACHE["nc"] = _build_program()
    return _CACHE["nc"]


def run_sharded(in_maps, trace=False):
    from concourse.bass_utils import run_bass_kernel_spmd
    nc = _get_program()
    return run_bass_kernel_spmd(nc, in_maps, list(range(NC_)), trace=trace)


def kernel(x, w_qkv, w_proj, attn_mask):
    in_maps = _host_inputs(x, w_qkv, w_proj, attn_mask)
    res = run_sharded(in_maps)
    acc = res.results[0]["out"].astype(np.float32)
    for c in range(1, NC_):
        acc = acc + res.results[c]["out"].astype(np.float32)
    return acc.reshape(1, T, C)


# revision 38
# speedup vs baseline: 1.1223x; 1.1223x over previous
"""Trainium2 Bass kernel for nn_MHA_2516850835986.

MHA: B=1, T=2048, C=2048, H=32 heads, d=64, causal, RoPE (head-indexed
angle quirk: within head h all feature pairs rotate by t * 10000^(-h/32)).

Sharding: head-parallel across 8 cores (4 heads each). x is replicated
(pre-transposed + pre-tiled on host), qkv columns / proj rows sharded by
head. Each core produces a partial [T, C] output (proj contraction over
its own heads' features) in bf16; partials are summed on host in f32.

v2 (from 409us baseline):
- bf16 operands everywhere (same 1 cycle/row PE rate as f32r at wide
  moving dims, but half the HBM traffic, half the SBUF footprint, and no
  f32r 4x penalty on narrow moving dims).
- software pipelining: tile i's proj + softmax-normalize tails are
  emitted AFTER tile i+1's qkv/rope matmuls, so the PE never waits on
  the reciprocal chain.
- reciprocal_approx_fast (~5x faster than reciprocal, 18 good bits).
- causal triangle: diagonal 128-blocks restrict S/exp/PV to t >= 128*b.
- RoPE cos-multiply moved to GpSimd; per-core layout pre-tiled on host
  so every input DMA is contiguous per partition.
"""

import sys

sys.path.insert(0, "/opt/trn_rl_repo")
import numpy as np

T = 2048
C = 2048
NH = 32          # total heads
HL = 4           # heads per core
D = 64           # head dim
NC_ = 8          # cores
TT = 512         # t-tile width
NTT = T // TT    # 4 t-tiles
KC = C // 128    # 16 contraction chunks (also 16 s-blocks of 128)
ROPE_THETA = 10000.0

_CACHE = {}


def _build_program():
    import concourse.bass as bass
    import concourse.tile as tile
    from concourse import bacc, mybir
    from contextlib import ExitStack

    F32 = mybir.dt.float32
    F32R = mybir.dt.float32r
    BF16 = mybir.dt.bfloat16
    EXP = mybir.ActivationFunctionType.Exp
    MUL = mybir.AluOpType.mult
    ADD = mybir.AluOpType.add

    nc = bacc.Bacc(None, target_bir_lowering=False)

    # host-pretiled x^T: [p, tile, kc, t'] so tile loads are contiguous
    xt = nc.declare_dram_parameter("xt", [128, NTT, KC, TT], BF16, False)
    wqk = nc.declare_dram_parameter("wqk", [128, KC, 512], BF16, False)
    wv = nc.declare_dram_parameter("wv", [128, KC, 256], BF16, False)
    wproj = nc.declare_dram_parameter("wproj", [128, 2, T], BF16, False)
    costab = nc.declare_dram_parameter("costab", [128, 2, T], BF16, False)
    sintab = nc.declare_dram_parameter("sintab", [128, 2, T], BF16, False)
    tri = nc.declare_dram_parameter("tri", [128, 4, TT], BF16, False)
    perm = nc.declare_dram_parameter("perm", [128, 128], F32R, False)
    eye = nc.declare_dram_parameter("eye", [64, 64], BF16, False)
    out = nc.declare_dram_parameter("out", [T, T], BF16, True)

    with tile.TileContext(nc) as tc, ExitStack() as ctx:
        consts = ctx.enter_context(tc.tile_pool(name="consts", bufs=1))
        xtp = ctx.enter_context(tc.tile_pool(name="xtp", bufs=8))
        csp = ctx.enter_context(tc.tile_pool(name="csp", bufs=4))
        qrawp = ctx.enter_context(tc.tile_pool(name="qrawp", bufs=1))
        csfp = ctx.enter_context(tc.tile_pool(name="csfp", bufs=1))
        rscp = ctx.enter_context(tc.tile_pool(name="rscp", bufs=2))
        qrotp = ctx.enter_context(tc.tile_pool(name="qrotp", bufs=2))
        persist = ctx.enter_context(tc.tile_pool(name="persist", bufs=1))
        p4p = ctx.enter_context(tc.tile_pool(name="p4p", bufs=2))
        ytp = ctx.enter_context(tc.tile_pool(name="ytp", bufs=2))
        ytmpp = ctx.enter_context(tc.tile_pool(name="ytmpp", bufs=2))
        ymp = ctx.enter_context(tc.tile_pool(name="ymp", bufs=4))
        rp = ctx.enter_context(tc.tile_pool(name="rp", bufs=1))
        outp = ctx.enter_context(tc.tile_pool(name="outp", bufs=8))

        # PSUM: S2 pairs / qk accum (2 banks x2) + y (1 bank x2) + misc (1 bank x2)
        sps = ctx.enter_context(tc.tile_pool(name="sps", bufs=2, space="PSUM"))
        yps = ctx.enter_context(tc.tile_pool(name="yps", bufs=2, space="PSUM"))
        unips = ctx.enter_context(tc.tile_pool(name="unips", bufs=2, space="PSUM"))

        wqk_sb = consts.tile([128, KC, 512], BF16)
        wv_sb = consts.tile([128, KC, 256], BF16)
        wproj_sb = consts.tile([128, 2, T], BF16)
        tri_sb = consts.tile([128, 4, TT], BF16)
        perm_sb = consts.tile([128, 128], F32R)
        eye_sb = consts.tile([64, 64], BF16)
        ones_sb = consts.tile([1, 64], F32R)
        nc.vector.memset(ones_sb[:].bitcast(F32), 1.0)

        # v in normal layout [s, dd]: per s-block slot of 4 heads x (64 v + 1 one + 1 pad)
        v_sb = persist.tile([128, KC, HL, 66], BF16)
        # fill everything with 1.0 once; v-copies overwrite cols 0:64 of each
        # slot, leaving col 64 as the ones-column for the denominator trick
        nc.vector.memset(v_sb[:].rearrange("p a b c -> p (a b c)"), 1.0)
        # k^T (rope'd), persistent across tiles: [dd(2 heads), block, t]
        krot = persist.tile([128, 2, T], BF16)

        def load_tile(j):
            """Issue input DMAs for t-tile j (sync HWDGE queue only).

            cos/sin arrive bf16 and are cast to f32 working tiles at use
            time (in qkvrope) to keep RoPE math in f32."""
            tslj = slice(TT * j, TT * (j + 1))
            xth = []
            for half in range(2):
                xh = xtp.tile([128, KC // 2, TT], BF16, tag="xt")
                nc.sync.dma_start(xh[:], xt[:, j, (KC // 2) * half:(KC // 2) * (half + 1), :])
                xth.append(xh)
            cos_t = csp.tile([128, 2, TT], BF16, tag="cos")
            nc.sync.dma_start(cos_t[:], costab[:, :, tslj])
            sin_t = csp.tile([128, 2, TT], BF16, tag="sin")
            nc.sync.dma_start(sin_t[:], sintab[:, :, tslj])
            return xth, cos_t, sin_t

        # tile-0 inputs interleaved with the wqk chunks so the first qk
        # chain can start after ~0.25MB instead of the whole preamble
        xh0 = xtp.tile([128, KC // 2, TT], BF16, tag="xt")
        xh1 = xtp.tile([128, KC // 2, TT], BF16, tag="xt")
        xhv = [xh0, xh1]
        for lo, hi in ((0, 2), (2, 4), (4, 8), (8, 12), (12, 16)):
            nc.sync.dma_start(wqk_sb[:, lo:hi, :], wqk[:, lo:hi, :])
            nc.sync.dma_start(xhv[lo // 8][:, lo % 8:lo % 8 + (hi - lo), :],
                              xt[:, 0, lo:hi, :])
        cos0 = csp.tile([128, 2, TT], BF16, tag="cos")
        nc.sync.dma_start(cos0[:], costab[:, :, 0:TT])
        sin0 = csp.tile([128, 2, TT], BF16, tag="sin")
        nc.sync.dma_start(sin0[:], sintab[:, :, 0:TT])
        nc.sync.dma_start(wv_sb[:], wv[:])
        nc.sync.dma_start(perm_sb[:], perm[:])
        nc.sync.dma_start(eye_sb[:], eye[:])
        nc.sync.dma_start(tri_sb[:], tri[:])
        # preload ALL remaining tiles' inputs up front (fits in SBUF at
        # bf16); ordered by first-use time, wproj between xt1 and xt2.
        # Steady state then has NO input DMA bursts competing with the
        # latency-critical yt-swap SBUF DMAs at tile boundaries.
        loads = [([xh0, xh1], cos0, sin0)]
        loads.append(load_tile(1))
        nc.sync.dma_start(wproj_sb[:], wproj[:])
        loads.append(load_tile(2))
        loads.append(load_tile(3))

        def qkvrope(i, xth, cos_t, sin_t):
            """qk/v matmuls + RoPE for tile i. Returns qrot; writes krot, v_sb.

            qk accumulation is quarter-interleaved (qq outer) so tile-0's
            first matmuls only wait on the first wqk/xt quarter DMA pair.
            The 4 m-psums live in the two 2-bank sps tiles as halves."""
            tsl = slice(TT * i, TT * (i + 1))
            psA = sps.tile([128, 2 * TT], F32, tag="S")
            psB = sps.tile([128, 2 * TT], F32, tag="S")
            mslot = [psA[:, 0:TT], psA[:, TT:2 * TT],
                     psB[:, 0:TT], psB[:, TT:2 * TT]]
            for qq in range(4):
                for m in range(4):
                    for kc in range(4 * qq, 4 * (qq + 1)):
                        nc.tensor.matmul(mslot[m],
                                         wqk_sb[:, kc, 128 * m:128 * (m + 1)],
                                         xth[kc // 8][:, kc % 8, :],
                                         start=(kc == 0), stop=(kc == KC - 1))
            qraw = qrawp.tile([128, 4, TT], F32R, tag="qraw")
            for m in range(4):
                nc.vector.tensor_copy(qraw[:, m, :], mslot[m])

            # cast bf16 cos/sin to f32 working tiles (keeps RoPE math f32);
            # on DVE — gpsimd CAST is ~5x slower and thrashes its library
            cs32 = csfp.tile([128, 4, TT], F32, tag="cs32")
            nc.vector.tensor_copy(cs32[:, 0:2, :], cos_t[:])
            nc.vector.tensor_copy(cs32[:, 2:4, :], sin_t[:])

            # ---- RoPE on q (blocks 0,1) and k (blocks 2,3) ----
            # dst = cos*src + sin*(perm@src); cos-mul on GpSimd, rest DVE;
            # the v block AFTER this keeps the PE busy while the DVE chain
            # finishes, so attention's S matmuls find qrot/krot ready
            qrot = qrotp.tile([128, 2, TT], BF16, tag="qrot")
            for bb in range(4):
                blk = bb % 2
                src = qraw[:, bb, :]
                dst = qrot[:, blk, :] if bb < 2 else krot[:, blk, tsl]
                psw = unips.tile([128, TT], F32, tag="uni")
                nc.tensor.matmul(psw[:], perm_sb[:], src, start=True, stop=True)
                nc.vector.tensor_tensor(psw[:], psw[:], cs32[:, 2 + blk, :], MUL)
                t1 = rscp.tile([128, TT], F32, tag="rsc")
                nc.gpsimd.tensor_tensor(t1[:], src.bitcast(F32), cs32[:, blk, :], MUL)
                nc.vector.tensor_tensor(dst, t1[:], psw[:], ADD)

            # ---- v matmuls (normal layout) ----
            for tc4 in range(4):
                psv = unips.tile([128, TT], F32, tag="uni")
                for kc in range(KC):
                    nc.tensor.matmul(psv[:, 0:256],
                                     xth[kc // 8][:, kc % 8, 128 * tc4:128 * (tc4 + 1)],
                                     wv_sb[:, kc, :],
                                     start=(kc == 0), stop=(kc == KC - 1))
                nc.vector.tensor_copy(
                    v_sb[:, 4 * i + tc4, :, 0:64],
                    psv[:, 0:256].rearrange("p (h d) -> p h d", h=HL))
            return qrot

        def finish_tails(yt, tails):
            """Broadcast each head's 1/denominator across 64 partitions via
            PE, normalize into yt (bf16); hh=1 is lifted onto partitions
            64:128 by an identity matmul into PSUM (no DMA — stays on the
            engines, so proj's dependency chain is short and local)."""
            for bp, hh, ym64, rsb in tails:
                psb = yps.tile([128, TT], F32, tag="y")
                nc.tensor.matmul(psb[0:64, :], ones_sb[:], rsb[:],
                                 start=True, stop=True)
                if hh == 0:
                    nc.vector.tensor_tensor(yt[0:64, bp, :], ym64[:],
                                            psb[0:64, :], MUL)
                else:
                    ytm = ytmpp.tile([64, TT], BF16, tag="ytmp2")
                    nc.vector.tensor_tensor(ytm[:], ym64[:], psb[0:64, :], MUL)
                    pmv = yps.tile([128, TT], F32, tag="y")
                    nc.tensor.matmul(pmv[64:128, :], eye_sb[:], ytm[:],
                                     start=True, stop=True)
                    nc.vector.tensor_copy(yt[64:128, bp, :], pmv[64:128, :])

        def attention(i, qrot):
            """S -> exp -> (mask) -> PV per head pair; evacuate y + start
            reciprocals. bp0's broadcast+normalize tails are emitted at the
            end (hidden behind bp1's matmuls); bp1's are returned for
            deferred emission (after the next tile's qkvrope)."""
            yt = ytp.tile([128, 2, TT], BF16, tag="yt")
            nsb = 4 * (i + 1)
            tails = []
            for bp in range(2):
                psyA = yps.tile([65, TT], F32, tag="y")
                psyB = yps.tile([65, TT], F32, tag="y")

                def emit_pv(sb, p4):
                    diag = sb - 4 * i
                    c0 = 128 * diag if diag > 0 else 0
                    nc.tensor.matmul(psyA[:, c0:TT], v_sb[:, sb, 2 * bp, 0:65],
                                     p4[:, c0:TT],
                                     start=(sb == 0), stop=(sb == nsb - 1))
                    nc.tensor.matmul(psyB[:, c0:TT], v_sb[:, sb, 2 * bp + 1, 0:65],
                                     p4[:, TT + c0:2 * TT],
                                     start=(sb == 0), stop=(sb == nsb - 1))

                prev = None   # (sb, p4) — PV trails S by one block to hide exp
                for sb in range(nsb):
                    diag = sb - 4 * i        # >= 0 on the diagonal t-tile
                    c0 = 128 * diag if diag > 0 else 0
                    s2 = sps.tile([128, 2 * TT], F32, tag="S")
                    nc.tensor.matmul(s2[:, c0:TT],
                                     krot[0:64, bp, 128 * sb:128 * (sb + 1)],
                                     qrot[0:64, bp, c0:TT],
                                     start=True, stop=True, tile_position=(0, 0))
                    nc.tensor.matmul(s2[:, TT + c0:2 * TT],
                                     krot[64:128, bp, 128 * sb:128 * (sb + 1)],
                                     qrot[64:128, bp, c0:TT],
                                     start=True, stop=True, tile_position=(64, 0))
                    p4 = p4p.tile([128, 2 * TT], BF16, tag="P4")
                    if c0 == 0:
                        nc.scalar.activation(p4[:], s2[:], EXP, scale=0.125)
                    else:
                        nc.scalar.activation(p4[:, c0:TT], s2[:, c0:TT],
                                             EXP, scale=0.125)
                        nc.scalar.activation(p4[:, TT + c0:2 * TT],
                                             s2[:, TT + c0:2 * TT],
                                             EXP, scale=0.125)
                    if diag >= 0:
                        # mask only the partial-triangle 128-col slab
                        for hh in range(2):
                            off = TT * hh + c0
                            nc.gpsimd.tensor_tensor(
                                p4[:, off:off + 128],
                                p4[:, off:off + 128],
                                tri_sb[:, diag, c0:c0 + 128], MUL)
                    if prev is not None:
                        emit_pv(*prev)
                    prev = (sb, p4)
                emit_pv(*prev)
                for hh, psy in ((0, psyA), (1, psyB)):
                    ym64 = ymp.tile([64, TT], F32, tag="ym")
                    nc.scalar.copy(ym64[:], psy[0:64, :])
                    # denominator row evacuated to partition 0 so the fast
                    # approx reciprocal works (it mishandles offset inputs)
                    dn0 = rp.tile([1, TT], F32, tag="dn", bufs=2)
                    nc.scalar.copy(dn0[:], psy[64:65, :])
                    rsf = rp.tile([1, TT], F32, tag="rsf", bufs=2)
                    rsb = rp.tile([1, TT], F32R, tag=f"r{bp}{hh}")
                    with nc.allow_low_precision("softmax recip approx+f32r"):
                        nc.vector.reciprocal_approx_fast(out=rsf[:], in_=dn0[:])
                        nc.vector.tensor_copy(rsb[:], rsf[:])
                    tails.append((bp, hh, ym64, rsb))
            # bp0's tails now: its reciprocals completed during bp1's matmuls
            finish_tails(yt, tails[0:2])
            return yt, tails[2:4]

        def emit_proj(j, ytj):
            """Partial out rows for t-tile j from its normalized y^T.
            Out DMAs round-robin over 3 queues (sync joins once the input
            preloads are done after tile 0)."""
            qs = ([nc.scalar.dma_start, nc.gpsimd.dma_start] if j == 0 else
                  [nc.scalar.dma_start, nc.gpsimd.dma_start,
                   nc.sync.dma_start])
            n = 0
            for tc4 in range(4):
                for ct in range(4):
                    pso = yps.tile([128, TT], F32, tag="y")
                    for b in range(2):
                        nc.tensor.matmul(pso[:],
                                         ytj[:, b, 128 * tc4:128 * (tc4 + 1)],
                                         wproj_sb[:, b, TT * ct:TT * (ct + 1)],
                                         start=(b == 0), stop=(b == 1))
                    osb = outp.tile([128, TT], BF16, tag="osb")
                    if ct % 2 == 0:
                        nc.scalar.copy(osb[:], pso[:])
                    else:
                        nc.vector.tensor_copy(osb[:], pso[:])
                    dst = out[TT * j + 128 * tc4: TT * j + 128 * (tc4 + 1),
                              TT * ct:TT * (ct + 1)]
                    qs[n % len(qs)](dst, osb[:])
                    n += 1

        qrots = {0: qkvrope(0, *loads[0])}
        for i in range(NTT):
            yt, tails = attention(i, qrots.pop(i))
            if i + 1 < NTT:
                qrots[i + 1] = qkvrope(i + 1, *loads[i + 1])
            finish_tails(yt, tails)
            emit_proj(i, yt)

    nc.finalize()
    return nc


def _host_inputs(x, w_qkv, w_proj, attn_mask):
    """Build the 8 per-core input maps (host-side sharding/layout prep)."""
    import ml_dtypes
    BF = ml_dtypes.bfloat16

    x = np.asarray(x)
    w_qkv = np.asarray(w_qkv)
    w_proj = np.asarray(w_proj)
    attn_mask = np.asarray(attn_mask)

    xT = x.reshape(T, C).T                               # [C, T] f32
    # pre-tiled [p, tile, kc, t'] contiguous bf16
    xtt = xT.reshape(KC, 128, NTT, TT).transpose(1, 2, 0, 3).astype(BF)

    # RoPE tables, faithful to the reference broadcasting quirk:
    # head g rotates all pairs by angle t * theta^(-g/32) (f32 math).
    inv_freq = (1.0 / (ROPE_THETA ** (np.arange(0, D, 2, dtype=np.float32) / D))
                ).astype(np.float32)                     # [32] indexed by head
    t_ar = np.arange(T, dtype=np.float32)
    freqs = (t_ar[:, None] * inv_freq[None, :]).astype(np.float32)  # [T, 32]
    cosf = np.cos(freqs).astype(np.float32)              # [T, 32]
    sinf = np.sin(freqs).astype(np.float32)
    sgn = np.where(np.arange(64) % 2 == 0, np.float32(-1.0), np.float32(1.0))  # [64]

    # 0/1 causal keep-masks from the actual mask, one per 128-row s-block of a
    # 512-wide diagonal t-tile: tri[s, b, t] = exp(mask[t, 128b + s])
    tri = np.empty((128, 4, TT), dtype=np.float32)
    for bq in range(4):
        tri[:, bq, :] = np.exp(
            attn_mask[0:TT, 128 * bq:128 * (bq + 1)].astype(np.float64)).T
    tri = tri.astype(BF)

    permM = np.zeros((128, 128), dtype=np.float32)
    permM[np.arange(128), np.arange(128) ^ 1] = 1.0

    in_maps = []
    for c in range(NC_):
        wqk_c = np.concatenate(
            [w_qkv[:, 256 * c:256 * (c + 1)],
             w_qkv[:, 2048 + 256 * c:2048 + 256 * (c + 1)]], axis=1)  # [C, 512]
        wqk_c = wqk_c.reshape(KC, 128, 512).transpose(1, 0, 2).astype(BF)
        wv_c = w_qkv[:, 4096 + 256 * c:4096 + 256 * (c + 1)]          # [C, 256]
        wv_c = wv_c.reshape(KC, 128, 256).transpose(1, 0, 2).astype(BF)
        wproj_c = w_proj[256 * c:256 * (c + 1), :]                    # [256, T]
        wproj_c = wproj_c.reshape(2, 128, T).transpose(1, 0, 2).astype(BF)

        costab = np.empty((128, 2, T), dtype=np.float32)
        sintab = np.empty((128, 2, T), dtype=np.float32)
        for bb in range(2):
            for p in range(128):
                g = 4 * c + 2 * bb + (p // 64)           # global head
                costab[p, bb, :] = cosf[:, g]
                sintab[p, bb, :] = sgn[p % 64] * sinf[:, g]
        costab = costab.astype(BF)
        sintab = sintab.astype(BF)

        in_maps.append({
            "xt": xtt, "wqk": wqk_c, "wv": wv_c, "wproj": wproj_c,
            "costab": costab, "sintab": sintab, "tri": tri, "perm": permM,
            "eye": np.eye(64, dtype=np.float32).astype(BF),
        })
    return in_maps


def _get_program():
    if "nc" not in _CACHE:
        _CACHE["nc"] = _build_program()
    return _CACHE["nc"]


def run_sharded(in_maps, trace=False):
    from concourse.bass_utils import run_bass_kernel_spmd
    nc = _get_program()
    return run_bass_kernel_spmd(nc, in_maps, list(range(NC_)), trace=trace)


def kernel(x, w_qkv, w_proj, attn_mask):
    in_maps = _host_inputs(x, w_qkv, w_proj, attn_mask)
    res = run_sharded(in_maps)
    acc = res.results[0]["out"].astype(np.float32)
    for c in range(1, NC_):
        acc = acc + res.results[c]["out"].astype(np.float32)
    return acc.reshape(1, T, C)


# revision 39
# speedup vs baseline: 1.1268x; 1.0040x over previous
"""Trainium2 Bass kernel for nn_MHA_2516850835986.

MHA: B=1, T=2048, C=2048, H=32 heads, d=64, causal, RoPE (head-indexed
angle quirk: within head h all feature pairs rotate by t * 10000^(-h/32)).

Sharding: head-parallel across 8 cores (4 heads each). x is replicated
(pre-transposed + pre-tiled on host), qkv columns / proj rows sharded by
head. Each core produces a partial [T, C] output (proj contraction over
its own heads' features) in bf16; partials are summed on host in f32.

v2 (from 409us baseline):
- bf16 operands everywhere (same 1 cycle/row PE rate as f32r at wide
  moving dims, but half the HBM traffic, half the SBUF footprint, and no
  f32r 4x penalty on narrow moving dims).
- software pipelining: tile i's proj + softmax-normalize tails are
  emitted AFTER tile i+1's qkv/rope matmuls, so the PE never waits on
  the reciprocal chain.
- reciprocal_approx_fast (~5x faster than reciprocal, 18 good bits).
- causal triangle: diagonal 128-blocks restrict S/exp/PV to t >= 128*b.
- RoPE cos-multiply moved to GpSimd; per-core layout pre-tiled on host
  so every input DMA is contiguous per partition.
"""

import sys

sys.path.insert(0, "/opt/trn_rl_repo")
import numpy as np

T = 2048
C = 2048
NH = 32          # total heads
HL = 4           # heads per core
D = 64           # head dim
NC_ = 8          # cores
TT = 512         # t-tile width
NTT = T // TT    # 4 t-tiles
KC = C // 128    # 16 contraction chunks (also 16 s-blocks of 128)
ROPE_THETA = 10000.0

_CACHE = {}


def _build_program():
    import concourse.bass as bass
    import concourse.tile as tile
    from concourse import bacc, mybir
    from contextlib import ExitStack

    F32 = mybir.dt.float32
    F32R = mybir.dt.float32r
    BF16 = mybir.dt.bfloat16
    EXP = mybir.ActivationFunctionType.Exp
    MUL = mybir.AluOpType.mult
    ADD = mybir.AluOpType.add

    nc = bacc.Bacc(None, target_bir_lowering=False)

    # host-pretiled x^T: [p, tile, kc, t'] so tile loads are contiguous
    xt = nc.declare_dram_parameter("xt", [128, NTT, KC, TT], BF16, False)
    wqk = nc.declare_dram_parameter("wqk", [128, KC, 512], BF16, False)
    wv = nc.declare_dram_parameter("wv", [128, KC, 256], BF16, False)
    wproj = nc.declare_dram_parameter("wproj", [128, 2, T], BF16, False)
    costab = nc.declare_dram_parameter("costab", [128, 2, T], BF16, False)
    sintab = nc.declare_dram_parameter("sintab", [128, 2, T], BF16, False)
    tri = nc.declare_dram_parameter("tri", [128, 4, TT], BF16, False)
    perm = nc.declare_dram_parameter("perm", [128, 128], F32R, False)
    eye = nc.declare_dram_parameter("eye", [64, 64], BF16, False)
    out = nc.declare_dram_parameter("out", [T, T], BF16, True)

    with tile.TileContext(nc) as tc, ExitStack() as ctx:
        consts = ctx.enter_context(tc.tile_pool(name="consts", bufs=1))
        xtp = ctx.enter_context(tc.tile_pool(name="xtp", bufs=8))
        csp = ctx.enter_context(tc.tile_pool(name="csp", bufs=4))
        qrawp = ctx.enter_context(tc.tile_pool(name="qrawp", bufs=1))
        csfp = ctx.enter_context(tc.tile_pool(name="csfp", bufs=1))
        rscp = ctx.enter_context(tc.tile_pool(name="rscp", bufs=2))
        qrotp = ctx.enter_context(tc.tile_pool(name="qrotp", bufs=2))
        persist = ctx.enter_context(tc.tile_pool(name="persist", bufs=1))
        p4p = ctx.enter_context(tc.tile_pool(name="p4p", bufs=2))
        ytp = ctx.enter_context(tc.tile_pool(name="ytp", bufs=2))
        ytmpp = ctx.enter_context(tc.tile_pool(name="ytmpp", bufs=2))
        ymp = ctx.enter_context(tc.tile_pool(name="ymp", bufs=4))
        rp = ctx.enter_context(tc.tile_pool(name="rp", bufs=1))
        outp = ctx.enter_context(tc.tile_pool(name="outp", bufs=8))

        # PSUM: S2 pairs / qk accum (2 banks x2) + y (1 bank x2) + misc (1 bank x2)
        sps = ctx.enter_context(tc.tile_pool(name="sps", bufs=2, space="PSUM"))
        yps = ctx.enter_context(tc.tile_pool(name="yps", bufs=2, space="PSUM"))
        unips = ctx.enter_context(tc.tile_pool(name="unips", bufs=2, space="PSUM"))

        wqk_sb = consts.tile([128, KC, 512], BF16)
        wv_sb = consts.tile([128, KC, 256], BF16)
        wproj_sb = consts.tile([128, 2, T], BF16)
        tri_sb = consts.tile([128, 4, TT], BF16)
        perm_sb = consts.tile([128, 128], F32R)
        eye_sb = consts.tile([64, 64], BF16)
        ones_sb = consts.tile([1, 64], F32R)
        nc.vector.memset(ones_sb[:].bitcast(F32), 1.0)

        # v in normal layout [s, dd]: per s-block slot of 4 heads x (64 v + 1 one + 1 pad)
        v_sb = persist.tile([128, KC, HL, 66], BF16)
        # fill everything with 1.0 once; v-copies overwrite cols 0:64 of each
        # slot, leaving col 64 as the ones-column for the denominator trick
        nc.vector.memset(v_sb[:].rearrange("p a b c -> p (a b c)"), 1.0)
        # k^T (rope'd), persistent across tiles: [dd(2 heads), block, t]
        krot = persist.tile([128, 2, T], BF16)

        def load_tile(j):
            """Issue input DMAs for t-tile j (sync HWDGE queue only).

            cos/sin arrive bf16 and are cast to f32 working tiles at use
            time (in qkvrope) to keep RoPE math in f32."""
            tslj = slice(TT * j, TT * (j + 1))
            xth = []
            for half in range(2):
                xh = xtp.tile([128, KC // 2, TT], BF16, tag="xt")
                nc.sync.dma_start(xh[:], xt[:, j, (KC // 2) * half:(KC // 2) * (half + 1), :])
                xth.append(xh)
            cos_t = csp.tile([128, 2, TT], BF16, tag="cos")
            nc.sync.dma_start(cos_t[:], costab[:, :, tslj])
            sin_t = csp.tile([128, 2, TT], BF16, tag="sin")
            nc.sync.dma_start(sin_t[:], sintab[:, :, tslj])
            return xth, cos_t, sin_t

        # tile-0 inputs interleaved with the wqk chunks so the first qk
        # chain can start after ~0.25MB instead of the whole preamble
        xh0 = xtp.tile([128, KC // 2, TT], BF16, tag="xt")
        xh1 = xtp.tile([128, KC // 2, TT], BF16, tag="xt")
        xhv = [xh0, xh1]
        for lo, hi in ((0, 2), (2, 4), (4, 8), (8, 12), (12, 16)):
            nc.sync.dma_start(wqk_sb[:, lo:hi, :], wqk[:, lo:hi, :])
            nc.sync.dma_start(xhv[lo // 8][:, lo % 8:lo % 8 + (hi - lo), :],
                              xt[:, 0, lo:hi, :])
        cos0 = csp.tile([128, 2, TT], BF16, tag="cos")
        nc.sync.dma_start(cos0[:], costab[:, :, 0:TT])
        sin0 = csp.tile([128, 2, TT], BF16, tag="sin")
        nc.sync.dma_start(sin0[:], sintab[:, :, 0:TT])
        nc.sync.dma_start(wv_sb[:], wv[:])
        nc.sync.dma_start(perm_sb[:], perm[:])
        nc.sync.dma_start(eye_sb[:], eye[:])
        nc.sync.dma_start(tri_sb[:], tri[:])
        # preload ALL remaining tiles' inputs up front (fits in SBUF at
        # bf16); ordered by first-use time, wproj between xt1 and xt2.
        # Steady state then has NO input DMA bursts competing with the
        # latency-critical yt-swap SBUF DMAs at tile boundaries.
        loads = [([xh0, xh1], cos0, sin0)]
        loads.append(load_tile(1))
        nc.sync.dma_start(wproj_sb[:], wproj[:])
        loads.append(load_tile(2))
        loads.append(load_tile(3))

        def qkvrope(i, xth, cos_t, sin_t):
            """qk/v matmuls + RoPE for tile i. Returns qrot; writes krot, v_sb.

            qk accumulation is quarter-interleaved (qq outer) so tile-0's
            first matmuls only wait on the first wqk/xt quarter DMA pair.
            The 4 m-psums live in the two 2-bank sps tiles as halves."""
            tsl = slice(TT * i, TT * (i + 1))
            psA = sps.tile([128, 2 * TT], F32, tag="S")
            psB = sps.tile([128, 2 * TT], F32, tag="S")
            mslot = [psA[:, 0:TT], psA[:, TT:2 * TT],
                     psB[:, 0:TT], psB[:, TT:2 * TT]]
            for qq in range(4):
                for m in range(4):
                    for kc in range(4 * qq, 4 * (qq + 1)):
                        nc.tensor.matmul(mslot[m],
                                         wqk_sb[:, kc, 128 * m:128 * (m + 1)],
                                         xth[kc // 8][:, kc % 8, :],
                                         start=(kc == 0), stop=(kc == KC - 1))
            qraw = qrawp.tile([128, 4, TT], F32R, tag="qraw")
            for m in range(4):
                nc.vector.tensor_copy(qraw[:, m, :], mslot[m])

            # cast bf16 cos/sin to f32 working tiles (keeps RoPE math f32);
            # on the Scalar engine (idle here) so the DVE queue stays clear
            # for the qraw evacuations that feed the perm matmuls
            cs32 = csfp.tile([128, 4, TT], F32, tag="cs32")
            nc.scalar.copy(cs32[:, 0:2, :], cos_t[:])
            nc.scalar.copy(cs32[:, 2:4, :], sin_t[:])

            # ---- RoPE on q (blocks 0,1) and k (blocks 2,3) ----
            # dst = cos*src + sin*(perm@src); cos-mul on GpSimd, rest DVE;
            # the v block AFTER this keeps the PE busy while the DVE chain
            # finishes, so attention's S matmuls find qrot/krot ready
            qrot = qrotp.tile([128, 2, TT], BF16, tag="qrot")
            for bb in range(4):
                blk = bb % 2
                src = qraw[:, bb, :]
                dst = qrot[:, blk, :] if bb < 2 else krot[:, blk, tsl]
                psw = unips.tile([128, TT], F32, tag="uni")
                nc.tensor.matmul(psw[:], perm_sb[:], src, start=True, stop=True)
                nc.vector.tensor_tensor(psw[:], psw[:], cs32[:, 2 + blk, :], MUL)
                t1 = rscp.tile([128, TT], F32, tag="rsc")
                nc.gpsimd.tensor_tensor(t1[:], src.bitcast(F32), cs32[:, blk, :], MUL)
                nc.vector.tensor_tensor(dst, t1[:], psw[:], ADD)

            # ---- v matmuls (normal layout) ----
            for tc4 in range(4):
                psv = unips.tile([128, TT], F32, tag="uni")
                for kc in range(KC):
                    nc.tensor.matmul(psv[:, 0:256],
                                     xth[kc // 8][:, kc % 8, 128 * tc4:128 * (tc4 + 1)],
                                     wv_sb[:, kc, :],
                                     start=(kc == 0), stop=(kc == KC - 1))
                nc.vector.tensor_copy(
                    v_sb[:, 4 * i + tc4, :, 0:64],
                    psv[:, 0:256].rearrange("p (h d) -> p h d", h=HL))
            return qrot

        def finish_tails(yt, tails):
            """Broadcast each head's 1/denominator across 64 partitions via
            PE, normalize into yt (bf16); hh=1 is lifted onto partitions
            64:128 by an identity matmul into PSUM (no DMA — stays on the
            engines, so proj's dependency chain is short and local)."""
            for bp, hh, ym64, rsb in tails:
                psb = yps.tile([128, TT], F32, tag="y")
                nc.tensor.matmul(psb[0:64, :], ones_sb[:], rsb[:],
                                 start=True, stop=True)
                if hh == 0:
                    nc.vector.tensor_tensor(yt[0:64, bp, :], ym64[:],
                                            psb[0:64, :], MUL)
                else:
                    ytm = ytmpp.tile([64, TT], BF16, tag="ytmp2")
                    nc.vector.tensor_tensor(ytm[:], ym64[:], psb[0:64, :], MUL)
                    pmv = yps.tile([128, TT], F32, tag="y")
                    nc.tensor.matmul(pmv[64:128, :], eye_sb[:], ytm[:],
                                     start=True, stop=True)
                    nc.vector.tensor_copy(yt[64:128, bp, :], pmv[64:128, :])

        def attention(i, qrot):
            """S -> exp -> (mask) -> PV per head pair; evacuate y + start
            reciprocals. bp0's broadcast+normalize tails are emitted at the
            end (hidden behind bp1's matmuls); bp1's are returned for
            deferred emission (after the next tile's qkvrope)."""
            yt = ytp.tile([128, 2, TT], BF16, tag="yt")
            nsb = 4 * (i + 1)
            tails = []
            for bp in range(2):
                psyA = yps.tile([65, TT], F32, tag="y")
                psyB = yps.tile([65, TT], F32, tag="y")

                def emit_pv(sb, p4):
                    diag = sb - 4 * i
                    c0 = 128 * diag if diag > 0 else 0
                    nc.tensor.matmul(psyA[:, c0:TT], v_sb[:, sb, 2 * bp, 0:65],
                                     p4[:, c0:TT],
                                     start=(sb == 0), stop=(sb == nsb - 1))
                    nc.tensor.matmul(psyB[:, c0:TT], v_sb[:, sb, 2 * bp + 1, 0:65],
                                     p4[:, TT + c0:2 * TT],
                                     start=(sb == 0), stop=(sb == nsb - 1))

                prev = None   # (sb, p4) — PV trails S by one block to hide exp
                for sb in range(nsb):
                    diag = sb - 4 * i        # >= 0 on the diagonal t-tile
                    c0 = 128 * diag if diag > 0 else 0
                    s2 = sps.tile([128, 2 * TT], F32, tag="S")
                    nc.tensor.matmul(s2[:, c0:TT],
                                     krot[0:64, bp, 128 * sb:128 * (sb + 1)],
                                     qrot[0:64, bp, c0:TT],
                                     start=True, stop=True, tile_position=(0, 0))
                    nc.tensor.matmul(s2[:, TT + c0:2 * TT],
                                     krot[64:128, bp, 128 * sb:128 * (sb + 1)],
                                     qrot[64:128, bp, c0:TT],
                                     start=True, stop=True, tile_position=(64, 0))
                    p4 = p4p.tile([128, 2 * TT], BF16, tag="P4")
                    if c0 == 0:
                        nc.scalar.activation(p4[:], s2[:], EXP, scale=0.125)
                    else:
                        nc.scalar.activation(p4[:, c0:TT], s2[:, c0:TT],
                                             EXP, scale=0.125)
                        nc.scalar.activation(p4[:, TT + c0:2 * TT],
                                             s2[:, TT + c0:2 * TT],
                                             EXP, scale=0.125)
                    if diag >= 0:
                        # mask only the partial-triangle 128-col slab
                        for hh in range(2):
                            off = TT * hh + c0
                            nc.gpsimd.tensor_tensor(
                                p4[:, off:off + 128],
                                p4[:, off:off + 128],
                                tri_sb[:, diag, c0:c0 + 128], MUL)
                    if prev is not None:
                        emit_pv(*prev)
                    prev = (sb, p4)
                emit_pv(*prev)
                for hh, psy in ((0, psyA), (1, psyB)):
                    ym64 = ymp.tile([64, TT], F32, tag="ym")
                    nc.scalar.copy(ym64[:], psy[0:64, :])
                    # denominator row evacuated to partition 0 so the fast
                    # approx reciprocal works (it mishandles offset inputs)
                    dn0 = rp.tile([1, TT], F32, tag="dn", bufs=2)
                    nc.scalar.copy(dn0[:], psy[64:65, :])
                    rsf = rp.tile([1, TT], F32, tag="rsf", bufs=2)
                    rsb = rp.tile([1, TT], F32R, tag=f"r{bp}{hh}")
                    with nc.allow_low_precision("softmax recip approx+f32r"):
                        nc.vector.reciprocal_approx_fast(out=rsf[:], in_=dn0[:])
                        nc.vector.tensor_copy(rsb[:], rsf[:])
                    tails.append((bp, hh, ym64, rsb))
            # bp0's tails now: its reciprocals completed during bp1's matmuls
            finish_tails(yt, tails[0:2])
            return yt, tails[2:4]

        def emit_proj(j, ytj):
            """Partial out rows for t-tile j from its normalized y^T.
            Out DMAs round-robin over 3 queues (sync joins once the input
            preloads are done after tile 0)."""
            qs = ([nc.scalar.dma_start, nc.gpsimd.dma_start] if j == 0 else
                  [nc.scalar.dma_start, nc.gpsimd.dma_start,
                   nc.sync.dma_start])
            n = 0
            for tc4 in range(4):
                for ct in range(4):
                    pso = yps.tile([128, TT], F32, tag="y")
                    for b in range(2):
                        nc.tensor.matmul(pso[:],
                                         ytj[:, b, 128 * tc4:128 * (tc4 + 1)],
                                         wproj_sb[:, b, TT * ct:TT * (ct + 1)],
                                         start=(b == 0), stop=(b == 1))
                    osb = outp.tile([128, TT], BF16, tag="osb")
                    if ct % 2 == 0:
                        nc.scalar.copy(osb[:], pso[:])
                    else:
                        nc.vector.tensor_copy(osb[:], pso[:])
                    dst = out[TT * j + 128 * tc4: TT * j + 128 * (tc4 + 1),
                              TT * ct:TT * (ct + 1)]
                    qs[n % len(qs)](dst, osb[:])
                    n += 1

        qrots = {0: qkvrope(0, *loads[0])}
        for i in range(NTT):
            yt, tails = attention(i, qrots.pop(i))
            if i + 1 < NTT:
                qrots[i + 1] = qkvrope(i + 1, *loads[i + 1])
            finish_tails(yt, tails)
            emit_proj(i, yt)

    nc.finalize()
    return nc


def _host_inputs(x, w_qkv, w_proj, attn_mask):
    """Build the 8 per-core input maps (host-side sharding/layout prep)."""
    import ml_dtypes
    BF = ml_dtypes.bfloat16

    x = np.asarray(x)
    w_qkv = np.asarray(w_qkv)
    w_proj = np.asarray(w_proj)
    attn_mask = np.asarray(attn_mask)

    xT = x.reshape(T, C).T                               # [C, T] f32
    # pre-tiled [p, tile, kc, t'] contiguous bf16
    xtt = xT.reshape(KC, 128, NTT, TT).transpose(1, 2, 0, 3).astype(BF)

    # RoPE tables, faithful to the reference broadcasting quirk:
    # head g rotates all pairs by angle t * theta^(-g/32) (f32 math).
    inv_freq = (1.0 / (ROPE_THETA ** (np.arange(0, D, 2, dtype=np.float32) / D))
                ).astype(np.float32)                     # [32] indexed by head
    t_ar = np.arange(T, dtype=np.float32)
    freqs = (t_ar[:, None] * inv_freq[None, :]).astype(np.float32)  # [T, 32]
    cosf = np.cos(freqs).astype(np.float32)              # [T, 32]
    sinf = np.sin(freqs).astype(np.float32)
    sgn = np.where(np.arange(64) % 2 == 0, np.float32(-1.0), np.float32(1.0))  # [64]

    # 0/1 causal keep-masks from the actual mask, one per 128-row s-block of a
    # 512-wide diagonal t-tile: tri[s, b, t] = exp(mask[t, 128b + s])
    tri = np.empty((128, 4, TT), dtype=np.float32)
    for bq in range(4):
        tri[:, bq, :] = np.exp(
            attn_mask[0:TT, 128 * bq:128 * (bq + 1)].astype(np.float64)).T
    tri = tri.astype(BF)

    permM = np.zeros((128, 128), dtype=np.float32)
    permM[np.arange(128), np.arange(128) ^ 1] = 1.0

    in_maps = []
    for c in range(NC_):
        wqk_c = np.concatenate(
            [w_qkv[:, 256 * c:256 * (c + 1)],
             w_qkv[:, 2048 + 256 * c:2048 + 256 * (c + 1)]], axis=1)  # [C, 512]
        wqk_c = wqk_c.reshape(KC, 128, 512).transpose(1, 0, 2).astype(BF)
        wv_c = w_qkv[:, 4096 + 256 * c:4096 + 256 * (c + 1)]          # [C, 256]
        wv_c = wv_c.reshape(KC, 128, 256).transpose(1, 0, 2).astype(BF)
        wproj_c = w_proj[256 * c:256 * (c + 1), :]                    # [256, T]
        wproj_c = wproj_c.reshape(2, 128, T).transpose(1, 0, 2).astype(BF)

        costab = np.empty((128, 2, T), dtype=np.float32)
        sintab = np.empty((128, 2, T), dtype=np.float32)
        for bb in range(2):
            for p in range(128):
                g = 4 * c + 2 * bb + (p // 64)           # global head
                costab[p, bb, :] = cosf[:, g]
                sintab[p, bb, :] = sgn[p % 64] * sinf[:, g]
        costab = costab.astype(BF)
        sintab = sintab.astype(BF)

        in_maps.append({
            "xt": xtt, "wqk": wqk_c, "wv": wv_c, "wproj": wproj_c,
            "costab": costab, "sintab": sintab, "tri": tri, "perm": permM,
            "eye": np.eye(64, dtype=np.float32).astype(BF),
        })
    return in_maps


def _get_program():
    if "nc" not in _CACHE:
        _CACHE["nc"] = _build_program()
    return _CACHE["nc"]


def run_sharded(in_maps, trace=False):
    from concourse.bass_utils import run_bass_kernel_spmd
    nc = _get_program()
    return run_bass_kernel_spmd(nc, in_maps, list(range(NC_)), trace=trace)


def kernel(x, w_qkv, w_proj, attn_mask):
    in_maps = _host_inputs(x, w_qkv, w_proj, attn_mask)
    res = run_sharded(in_maps)
    acc = res.results[0]["out"].astype(np.float32)
    for c in range(1, NC_):
        acc = acc + res.results[c]["out"].astype(np.float32)
    return acc.reshape(1, T, C)


# revision 43
# speedup vs baseline: 1.2477x; 1.1073x over previous
"""Trainium2 Bass kernel for nn_MHA_2516850835986.

MHA: B=1, T=2048, C=2048, H=32 heads, d=64, causal, RoPE (head-indexed
angle quirk: within head h all feature pairs rotate by t * 10000^(-h/32)).

Sharding: head-parallel across 8 cores (4 heads each). x is replicated
(pre-transposed + pre-tiled on host), qkv columns / proj rows sharded by
head. Each core produces a partial [T, C] output (proj contraction over
its own heads' features) in bf16; partials are summed on host in f32.

v2 (from 409us baseline):
- bf16 operands everywhere (same 1 cycle/row PE rate as f32r at wide
  moving dims, but half the HBM traffic, half the SBUF footprint, and no
  f32r 4x penalty on narrow moving dims).
- software pipelining: tile i's proj + softmax-normalize tails are
  emitted AFTER tile i+1's qkv/rope matmuls, so the PE never waits on
  the reciprocal chain.
- reciprocal_approx_fast (~5x faster than reciprocal, 18 good bits).
- causal triangle: diagonal 128-blocks restrict S/exp/PV to t >= 128*b.
- RoPE cos-multiply moved to GpSimd; per-core layout pre-tiled on host
  so every input DMA is contiguous per partition.
"""

import sys

sys.path.insert(0, "/opt/trn_rl_repo")
import numpy as np

T = 2048
C = 2048
NH = 32          # total heads
HL = 4           # heads per core
D = 64           # head dim
NC_ = 8          # cores
TT = 512         # t-tile width
NTT = T // TT    # 4 t-tiles
KC = C // 128    # 16 contraction chunks (also 16 s-blocks of 128)
ROPE_THETA = 10000.0

_CACHE = {}


def _build_program():
    import concourse.bass as bass
    import concourse.tile as tile
    from concourse import bacc, mybir
    from contextlib import ExitStack

    F32 = mybir.dt.float32
    F32R = mybir.dt.float32r
    BF16 = mybir.dt.bfloat16
    EXP = mybir.ActivationFunctionType.Exp
    MUL = mybir.AluOpType.mult
    ADD = mybir.AluOpType.add

    nc = bacc.Bacc(None, target_bir_lowering=False)

    # host-pretiled x^T: [p, tile, kc, t'] so tile loads are contiguous
    xt = nc.declare_dram_parameter("xt", [128, NTT, KC, TT], BF16, False)
    wqk = nc.declare_dram_parameter("wqk", [128, KC, 512], BF16, False)
    wv = nc.declare_dram_parameter("wv", [128, KC, 256], BF16, False)
    wproj = nc.declare_dram_parameter("wproj", [128, 2, T], BF16, False)
    costab = nc.declare_dram_parameter("costab", [128, 2, T], BF16, False)
    sintab = nc.declare_dram_parameter("sintab", [128, 2, T], BF16, False)
    tri = nc.declare_dram_parameter("tri", [128, 4, TT], BF16, False)
    perm = nc.declare_dram_parameter("perm", [128, 128], BF16, False)
    eye = nc.declare_dram_parameter("eye", [64, 64], BF16, False)
    out = nc.declare_dram_parameter("out", [T, T], BF16, True)

    with tile.TileContext(nc) as tc, ExitStack() as ctx:
        consts = ctx.enter_context(tc.tile_pool(name="consts", bufs=1))
        xtp = ctx.enter_context(tc.tile_pool(name="xtp", bufs=8))
        csp = ctx.enter_context(tc.tile_pool(name="csp", bufs=4))
        qrawp = ctx.enter_context(tc.tile_pool(name="qrawp", bufs=1))
        csfp = ctx.enter_context(tc.tile_pool(name="csfp", bufs=1))
        rscp = ctx.enter_context(tc.tile_pool(name="rscp", bufs=2))
        qrotp = ctx.enter_context(tc.tile_pool(name="qrotp", bufs=2))
        persist = ctx.enter_context(tc.tile_pool(name="persist", bufs=1))
        p4p = ctx.enter_context(tc.tile_pool(name="p4p", bufs=2))
        ytp = ctx.enter_context(tc.tile_pool(name="ytp", bufs=2))
        ytmpp = ctx.enter_context(tc.tile_pool(name="ytmpp", bufs=2))
        ymp = ctx.enter_context(tc.tile_pool(name="ymp", bufs=4))
        rp = ctx.enter_context(tc.tile_pool(name="rp", bufs=1))
        outp = ctx.enter_context(tc.tile_pool(name="outp", bufs=8))

        # PSUM: S2 pairs / qk accum (2 banks x2) + y (1 bank x2) + misc (1 bank x2)
        sps = ctx.enter_context(tc.tile_pool(name="sps", bufs=2, space="PSUM"))
        yps = ctx.enter_context(tc.tile_pool(name="yps", bufs=2, space="PSUM"))
        unips = ctx.enter_context(tc.tile_pool(name="unips", bufs=2, space="PSUM"))

        wqk_sb = consts.tile([128, KC, 512], BF16)
        wv_sb = consts.tile([128, KC, 256], BF16)
        wproj_sb = consts.tile([128, 2, T], BF16)
        tri_sb = consts.tile([128, 4, TT], BF16)
        perm_sb = consts.tile([128, 128], BF16)
        eye_sb = consts.tile([64, 64], BF16)
        ones_sb = consts.tile([1, 64], F32R)
        nc.vector.memset(ones_sb[:].bitcast(F32), 1.0)

        # v in normal layout [s, dd]: per s-block slot of 4 heads x (64 v + 1 one + 1 pad)
        v_sb = persist.tile([128, KC, HL, 66], BF16)
        # fill everything with 1.0 once; v-copies overwrite cols 0:64 of each
        # slot, leaving col 64 as the ones-column for the denominator trick
        nc.vector.memset(v_sb[:].rearrange("p a b c -> p (a b c)"), 1.0)
        # k^T (rope'd), persistent across tiles: [dd(2 heads), block, t]
        krot = persist.tile([128, 2, T], BF16)

        def load_tile(j):
            """Issue input DMAs for t-tile j (sync HWDGE queue only).

            cos/sin arrive bf16 and are cast to f32 working tiles at use
            time (in qkvrope) to keep RoPE math in f32."""
            tslj = slice(TT * j, TT * (j + 1))
            xth = []
            for half in range(2):
                xh = xtp.tile([128, KC // 2, TT], BF16, tag="xt")
                nc.sync.dma_start(xh[:], xt[:, j, (KC // 2) * half:(KC // 2) * (half + 1), :])
                xth.append(xh)
            cos_t = csp.tile([128, 2, TT], BF16, tag="cos")
            nc.sync.dma_start(cos_t[:], costab[:, :, tslj])
            sin_t = csp.tile([128, 2, TT], BF16, tag="sin")
            nc.sync.dma_start(sin_t[:], sintab[:, :, tslj])
            return xth, cos_t, sin_t

        # tile-0 inputs interleaved with the wqk chunks so the first qk
        # chain can start after ~0.25MB instead of the whole preamble
        xh0 = xtp.tile([128, KC // 2, TT], BF16, tag="xt")
        xh1 = xtp.tile([128, KC // 2, TT], BF16, tag="xt")
        xhv = [xh0, xh1]
        for lo, hi in ((0, 2), (2, 4), (4, 8), (8, 12), (12, 16)):
            nc.sync.dma_start(wqk_sb[:, lo:hi, :], wqk[:, lo:hi, :])
            nc.sync.dma_start(xhv[lo // 8][:, lo % 8:lo % 8 + (hi - lo), :],
                              xt[:, 0, lo:hi, :])
        cos0 = csp.tile([128, 2, TT], BF16, tag="cos")
        nc.sync.dma_start(cos0[:], costab[:, :, 0:TT])
        sin0 = csp.tile([128, 2, TT], BF16, tag="sin")
        nc.sync.dma_start(sin0[:], sintab[:, :, 0:TT])
        nc.sync.dma_start(wv_sb[:], wv[:])
        nc.sync.dma_start(perm_sb[:], perm[:])
        nc.sync.dma_start(eye_sb[:], eye[:])
        nc.sync.dma_start(tri_sb[:], tri[:])
        # preload ALL remaining tiles' inputs up front (fits in SBUF at
        # bf16); ordered by first-use time, wproj between xt1 and xt2.
        # Steady state then has NO input DMA bursts competing with the
        # latency-critical yt-swap SBUF DMAs at tile boundaries.
        loads = [([xh0, xh1], cos0, sin0)]
        loads.append(load_tile(1))
        nc.sync.dma_start(wproj_sb[:], wproj[:])
        loads.append(load_tile(2))
        loads.append(load_tile(3))

        def qkvrope(i, xth, cos_t, sin_t):
            """qk/v matmuls + RoPE for tile i. Returns qrot; writes krot, v_sb.

            qk accumulation is quarter-interleaved (qq outer) so tile-0's
            first matmuls only wait on the first wqk/xt quarter DMA pair.
            The 4 m-psums live in the two 2-bank sps tiles as halves."""
            tsl = slice(TT * i, TT * (i + 1))
            psA = sps.tile([128, 2 * TT], F32, tag="S")
            psB = sps.tile([128, 2 * TT], F32, tag="S")
            mslot = [psA[:, 0:TT], psA[:, TT:2 * TT],
                     psB[:, 0:TT], psB[:, TT:2 * TT]]
            for qq in range(4):
                for m in range(4):
                    for kc in range(4 * qq, 4 * (qq + 1)):
                        nc.tensor.matmul(mslot[m],
                                         wqk_sb[:, kc, 128 * m:128 * (m + 1)],
                                         xth[kc // 8][:, kc % 8, :],
                                         start=(kc == 0), stop=(kc == KC - 1))
            qraw = qrawp.tile([128, 4, TT], BF16, tag="qraw")
            for m in range(4):
                nc.vector.tensor_copy(qraw[:, m, :], mslot[m])

            # only sin needs an f32 working copy (it multiplies f32 psum);
            # cast on the Scalar engine (idle here) so the DVE queue stays
            # clear for the qraw evacuations that feed the perm matmuls
            cs32 = csfp.tile([128, 2, TT], F32, tag="cs32")
            nc.scalar.copy(cs32[:], sin_t[:])

            # ---- RoPE (blocks 0,1=q, 2,3=k) interleaved with v matmuls ----
            # dst = cos*src + sin*(perm@src); cos-mul on GpSimd reads the
            # bf16 cos table directly; each v block keeps the PE busy while
            # the DVE/gpsimd chain of the preceding perm output drains
            qrot = qrotp.tile([128, 2, TT], BF16, tag="qrot")
            for bb in range(4):
                blk = bb % 2
                src = qraw[:, bb, :]
                dst = qrot[:, blk, :] if bb < 2 else krot[:, blk, tsl]
                psw = unips.tile([128, TT], F32, tag="uni")
                nc.tensor.matmul(psw[:], perm_sb[:], src, start=True, stop=True)
                nc.vector.tensor_tensor(psw[:], psw[:], cs32[:, blk, :], MUL)
                t1 = rscp.tile([128, TT], F32, tag="rsc")
                nc.gpsimd.tensor_tensor(t1[:], src, cos_t[:, blk, :], MUL)
                nc.vector.tensor_tensor(dst, t1[:], psw[:], ADD)

                # ---- v matmuls for block bb (normal layout) ----
                psv = unips.tile([128, TT], F32, tag="uni")
                for kc in range(KC):
                    nc.tensor.matmul(psv[:, 0:256],
                                     xth[kc // 8][:, kc % 8, 128 * bb:128 * (bb + 1)],
                                     wv_sb[:, kc, :],
                                     start=(kc == 0), stop=(kc == KC - 1))
                nc.scalar.copy(
                    v_sb[:, 4 * i + bb, :, 0:64],
                    psv[:, 0:256].rearrange("p (h d) -> p h d", h=HL))
            return qrot

        def finish_tails(yt, tails):
            """Broadcast each head's 1/denominator across 64 partitions via
            PE, normalize into yt (bf16); hh=1 is lifted onto partitions
            64:128 by an identity matmul into PSUM (no DMA — stays on the
            engines, so proj's dependency chain is short and local)."""
            for bp, hh, ym64, rsb in tails:
                psb = yps.tile([128, TT], F32, tag="y")
                nc.tensor.matmul(psb[0:64, :], ones_sb[:], rsb[:],
                                 start=True, stop=True)
                if hh == 0:
                    nc.vector.tensor_tensor(yt[0:64, bp, :], ym64[:],
                                            psb[0:64, :], MUL)
                else:
                    ytm = ytmpp.tile([64, TT], BF16, tag="ytmp2")
                    nc.vector.tensor_tensor(ytm[:], ym64[:], psb[0:64, :], MUL)
                    pmv = yps.tile([128, TT], F32, tag="y")
                    nc.tensor.matmul(pmv[64:128, :], eye_sb[:], ytm[:],
                                     start=True, stop=True)
                    nc.vector.tensor_copy(yt[64:128, bp, :], pmv[64:128, :])

        def attention(i, qrot):
            """S -> exp -> (mask) -> PV per head pair; evacuate y + start
            reciprocals. bp0's broadcast+normalize tails are emitted at the
            end (hidden behind bp1's matmuls); bp1's are returned for
            deferred emission (after the next tile's qkvrope)."""
            yt = ytp.tile([128, 2, TT], BF16, tag="yt")
            nsb = 4 * (i + 1)
            tails = []
            for bp in range(2):
                psyA = yps.tile([65, TT], F32, tag="y")
                psyB = yps.tile([65, TT], F32, tag="y")

                def emit_pv(sb, p4):
                    diag = sb - 4 * i
                    c0 = 128 * diag if diag > 0 else 0
                    nc.tensor.matmul(psyA[:, c0:TT], v_sb[:, sb, 2 * bp, 0:65],
                                     p4[:, c0:TT],
                                     start=(sb == 0), stop=(sb == nsb - 1))
                    nc.tensor.matmul(psyB[:, c0:TT], v_sb[:, sb, 2 * bp + 1, 0:65],
                                     p4[:, TT + c0:2 * TT],
                                     start=(sb == 0), stop=(sb == nsb - 1))

                prev = None   # (sb, p4) — PV trails S by one block to hide exp
                for sb in range(nsb):
                    diag = sb - 4 * i        # >= 0 on the diagonal t-tile
                    c0 = 128 * diag if diag > 0 else 0
                    s2 = sps.tile([128, 2 * TT], F32, tag="S")
                    nc.tensor.matmul(s2[:, c0:TT],
                                     krot[0:64, bp, 128 * sb:128 * (sb + 1)],
                                     qrot[0:64, bp, c0:TT],
                                     start=True, stop=True, tile_position=(0, 0))
                    nc.tensor.matmul(s2[:, TT + c0:2 * TT],
                                     krot[64:128, bp, 128 * sb:128 * (sb + 1)],
                                     qrot[64:128, bp, c0:TT],
                                     start=True, stop=True, tile_position=(64, 0))
                    p4 = p4p.tile([128, 2 * TT], BF16, tag="P4")
                    if c0 == 0:
                        nc.scalar.activation(p4[:], s2[:], EXP, scale=0.125)
                    else:
                        nc.scalar.activation(p4[:, c0:TT], s2[:, c0:TT],
                                             EXP, scale=0.125)
                        nc.scalar.activation(p4[:, TT + c0:2 * TT],
                                             s2[:, TT + c0:2 * TT],
                                             EXP, scale=0.125)
                    if diag >= 0:
                        # mask only the partial-triangle 128-col slab
                        for hh in range(2):
                            off = TT * hh + c0
                            nc.gpsimd.tensor_tensor(
                                p4[:, off:off + 128],
                                p4[:, off:off + 128],
                                tri_sb[:, diag, c0:c0 + 128], MUL)
                    if prev is not None:
                        emit_pv(*prev)
                    prev = (sb, p4)
                emit_pv(*prev)
                for hh, psy in ((0, psyA), (1, psyB)):
                    ym64 = ymp.tile([64, TT], F32, tag="ym")
                    nc.scalar.copy(ym64[:], psy[0:64, :])
                    # denominator row evacuated to partition 0 so the fast
                    # approx reciprocal works (it mishandles offset inputs)
                    dn0 = rp.tile([1, TT], F32, tag="dn", bufs=2)
                    nc.scalar.copy(dn0[:], psy[64:65, :])
                    rsf = rp.tile([1, TT], F32, tag="rsf", bufs=2)
                    rsb = rp.tile([1, TT], F32R, tag=f"r{bp}{hh}")
                    with nc.allow_low_precision("softmax recip approx+f32r"):
                        nc.vector.reciprocal_approx_fast(out=rsf[:], in_=dn0[:])
                        nc.vector.tensor_copy(rsb[:], rsf[:])
                    tails.append((bp, hh, ym64, rsb))
            # bp0's tails now: its reciprocals completed during bp1's matmuls
            finish_tails(yt, tails[0:2])
            return yt, tails[2:4]

        def emit_proj(j, ytj):
            """Partial out rows for t-tile j from its normalized y^T.
            Out DMAs round-robin over 3 queues (sync joins once the input
            preloads are done after tile 0)."""
            qs = ([nc.scalar.dma_start, nc.gpsimd.dma_start] if j == 0 else
                  [nc.scalar.dma_start, nc.gpsimd.dma_start,
                   nc.sync.dma_start])
            n = 0
            for tc4 in range(4):
                for ct in range(4):
                    pso = yps.tile([128, TT], F32, tag="y")
                    for b in range(2):
                        nc.tensor.matmul(pso[:],
                                         ytj[:, b, 128 * tc4:128 * (tc4 + 1)],
                                         wproj_sb[:, b, TT * ct:TT * (ct + 1)],
                                         start=(b == 0), stop=(b == 1))
                    osb = outp.tile([128, TT], BF16, tag="osb")
                    if ct % 2 == 0:
                        nc.scalar.copy(osb[:], pso[:])
                    else:
                        nc.vector.tensor_copy(osb[:], pso[:])
                    dst = out[TT * j + 128 * tc4: TT * j + 128 * (tc4 + 1),
                              TT * ct:TT * (ct + 1)]
                    qs[n % len(qs)](dst, osb[:])
                    n += 1

        qrots = {0: qkvrope(0, *loads[0])}
        for i in range(NTT):
            yt, tails = attention(i, qrots.pop(i))
            if i + 1 < NTT:
                qrots[i + 1] = qkvrope(i + 1, *loads[i + 1])
            finish_tails(yt, tails)
            emit_proj(i, yt)

    nc.finalize()
    return nc


def _host_inputs(x, w_qkv, w_proj, attn_mask):
    """Build the 8 per-core input maps (host-side sharding/layout prep)."""
    import ml_dtypes
    BF = ml_dtypes.bfloat16

    x = np.asarray(x)
    w_qkv = np.asarray(w_qkv)
    w_proj = np.asarray(w_proj)
    attn_mask = np.asarray(attn_mask)

    xT = x.reshape(T, C).T                               # [C, T] f32
    # pre-tiled [p, tile, kc, t'] contiguous bf16
    xtt = xT.reshape(KC, 128, NTT, TT).transpose(1, 2, 0, 3).astype(BF)

    # RoPE tables, faithful to the reference broadcasting quirk:
    # head g rotates all pairs by angle t * theta^(-g/32) (f32 math).
    inv_freq = (1.0 / (ROPE_THETA ** (np.arange(0, D, 2, dtype=np.float32) / D))
                ).astype(np.float32)                     # [32] indexed by head
    t_ar = np.arange(T, dtype=np.float32)
    freqs = (t_ar[:, None] * inv_freq[None, :]).astype(np.float32)  # [T, 32]
    cosf = np.cos(freqs).astype(np.float32)              # [T, 32]
    sinf = np.sin(freqs).astype(np.float32)
    sgn = np.where(np.arange(64) % 2 == 0, np.float32(-1.0), np.float32(1.0))  # [64]

    # 0/1 causal keep-masks from the actual mask, one per 128-row s-block of a
    # 512-wide diagonal t-tile: tri[s, b, t] = exp(mask[t, 128b + s])
    tri = np.empty((128, 4, TT), dtype=np.float32)
    for bq in range(4):
        tri[:, bq, :] = np.exp(
            attn_mask[0:TT, 128 * bq:128 * (bq + 1)].astype(np.float64)).T
    tri = tri.astype(BF)

    permM = np.zeros((128, 128), dtype=np.float32)
    permM[np.arange(128), np.arange(128) ^ 1] = 1.0
    permM = permM.astype(BF)

    in_maps = []
    for c in range(NC_):
        wqk_c = np.concatenate(
            [w_qkv[:, 256 * c:256 * (c + 1)],
             w_qkv[:, 2048 + 256 * c:2048 + 256 * (c + 1)]], axis=1)  # [C, 512]
        wqk_c = wqk_c.reshape(KC, 128, 512).transpose(1, 0, 2).astype(BF)
        wv_c = w_qkv[:, 4096 + 256 * c:4096 + 256 * (c + 1)]          # [C, 256]
        wv_c = wv_c.reshape(KC, 128, 256).transpose(1, 0, 2).astype(BF)
        wproj_c = w_proj[256 * c:256 * (c + 1), :]                    # [256, T]
        wproj_c = wproj_c.reshape(2, 128, T).transpose(1, 0, 2).astype(BF)

        costab = np.empty((128, 2, T), dtype=np.float32)
        sintab = np.empty((128, 2, T), dtype=np.float32)
        for bb in range(2):
            for p in range(128):
                g = 4 * c + 2 * bb + (p // 64)           # global head
                costab[p, bb, :] = cosf[:, g]
                sintab[p, bb, :] = sgn[p % 64] * sinf[:, g]
        costab = costab.astype(BF)
        sintab = sintab.astype(BF)

        in_maps.append({
            "xt": xtt, "wqk": wqk_c, "wv": wv_c, "wproj": wproj_c,
            "costab": costab, "sintab": sintab, "tri": tri, "perm": permM,
            "eye": np.eye(64, dtype=np.float32).astype(BF),
        })
    return in_maps


def _get_program():
    if "nc" not in _CACHE:
        _CACHE["nc"] = _build_program()
    return _CACHE["nc"]


def run_sharded(in_maps, trace=False):
    from concourse.bass_utils import run_bass_kernel_spmd
    nc = _get_program()
    return run_bass_kernel_spmd(nc, in_maps, list(range(NC_)), trace=trace)


def kernel(x, w_qkv, w_proj, attn_mask):
    in_maps = _host_inputs(x, w_qkv, w_proj, attn_mask)
    res = run_sharded(in_maps)
    acc = res.results[0]["out"].astype(np.float32)
    for c in range(1, NC_):
        acc = acc + res.results[c]["out"].astype(np.float32)
    return acc.reshape(1, T, C)
